# revision 21
# baseline (speedup 1.0000x reference)
"""NetVLAD pooling kernel for Trainium2 (Bass/Tile), 8-core data-parallel.

Reference computation (per batch b):
    scores = conv_w @ x[b]                  # [K, N]
    assign = softmax(scores, axis=K)
    vlad   = x[b] @ assign.T - centers * assign.sum(n)   # [D, K]
    vlad  /= max(||vlad||_2 over D, eps)    # intra-norm per cluster column
    desc   = vlad.reshape(D*K) / max(||.||_2, eps)

Shapes: x [32, 512, 1024] f32, conv_w [64, 512], centers [512, 64],
output desc [32, 32768] f32.  Sharding: data-parallel over batch,
4 batches per core; params replicated.

Per-core layout: batches are processed in PAIRS column-packed into the
128-wide PE array (batch b0 on array columns / output partitions 0:64,
b1 on 64:128 via tile_position), so most matmuls, activations, softmax
reduces and norm-chain ops handle two batches at once.  The matmul
path runs in fp16 (FWL weight loads, 2x DVE evacuations, pairing legal
— fp32r forbids nonzero dst partition); PSUM accumulation and the norm
chain stay fp32.  Each column-group accumulates into its OWN psum bank
(interleaved accumulation groups in one bank are unsafe because
start_tensor_calc's has_written clear granularity is bank-wide).

x is DMA'd and cast to fp16 in n-halves (1 MB granularity) so the
scores/softmax/x^T pipeline starts as soon as the first half-batches
arrive instead of waiting for whole batches.

Softmax normalization is folded into the x^T PSUM->SBUF evacuation
(scale=1/rowsum), so vlad consumes the UNNORMALIZED exp(scores)^T as
the stationary, and the assign row-sums come from an rhs=rec matmul
riding the same stationary.  1/sqrt(ss) is computed on DVE with the
bitcast fast-inverse-sqrt seed + one Newton step (~0.2% worst case,
well inside tolerance); ACT then only ever uses Exp/Square/Copy/
Identity, which share one table set - zero ACT table reloads.
(scalar_tensor_tensor and tensor_tensor_reduce fault this HW runtime -
NRT_EXEC_UNIT_UNRECOVERABLE - so the norm chain sticks to plain ops.)

The second L2 normalization is folded to a constant 1/8: after the
intra-normalization each of the K=64 columns has unit norm, so
||desc|| = 8 up to fp32 rounding.

Emission software-pipelines the two pairs: pair p's x^T transposes
interleave with pair p-1's vlad matmuls so the PE never idles long
enough for the HAM clock gate to re-throttle.
"""

import numpy as np

import concourse.bass as bass
from concourse import bacc
import concourse.mybir as mybir
import concourse.tile as tile
from concourse.bass_utils import run_bass_kernel_spmd
from concourse.masks import make_identity

B, D, K, N = 32, 512, 64, 1024
NCORES = 8
BC = B // NCORES          # batches per core
NPAIR = BC // 2           # batch pairs per core
F32 = mybir.dt.float32
F16 = mybir.dt.float16
I32 = mybir.dt.int32
EPS = 1e-12
DC = D // 128             # d chunks (4)
NB = N // 128             # n chunks (8)
NH = N // 2               # n elements per half

AF = mybir.ActivationFunctionType
ALU = mybir.AluOpType
AX = mybir.AxisListType

# fast-inverse-sqrt seed for rsqrt(2*h): C' = 0x5f3759df - 0x400000
_FISR_C = 0x5EF759DF


def _netvlad_core(ctx, tc, out, x, w, c):
    """Emit the per-core tile program.

    out: desc [BC, D*K] f32 DRAM     x: [BC, D, N] f32 DRAM
    w:   conv_w [K, D] f32 DRAM      c: centers [D, K] f32 DRAM
    """
    nc = tc.nc

    const = ctx.enter_context(tc.tile_pool(name="const", bufs=1))
    xpool = ctx.enter_context(tc.tile_pool(name="xp", bufs=4))
    x16p = ctx.enter_context(tc.tile_pool(name="x16", bufs=8))
    epool = ctx.enter_context(tc.tile_pool(name="ep", bufs=2))
    apool = ctx.enter_context(tc.tile_pool(name="ap", bufs=2))
    spool = ctx.enter_context(tc.tile_pool(name="sp", bufs=4))
    xtp = ctx.enter_context(tc.tile_pool(name="xtp", bufs=16))
    vpool = ctx.enter_context(tc.tile_pool(name="vp", bufs=2))
    opool = ctx.enter_context(tc.tile_pool(name="op", bufs=2))
    # PSUM 8 banks: s(2, also hosts as tiles) + et(1) + xt(2) + v(2) + o(1)
    ps_s = ctx.enter_context(tc.tile_pool(name="pss", bufs=2, space="PSUM"))
    ps_et = ctx.enter_context(tc.tile_pool(name="pset", bufs=1, space="PSUM"))
    ps_xt = ctx.enter_context(tc.tile_pool(name="psxt", bufs=2, space="PSUM"))
    ps_v = ctx.enter_context(tc.tile_pool(name="psv", bufs=2, space="PSUM"))
    ps_o = ctx.enter_context(tc.tile_pool(name="pso", bufs=1, space="PSUM"))

    # ---- constants ----------------------------------------------------
    # (w/c DMAs go out on the ACT hwdge queue so the x loads own sync's)
    ident = const.tile([128, 128], F32, tag="ident")
    make_identity(nc, ident)
    ident16 = const.tile([128, 128], F16, tag="ident16")
    make_identity(nc, ident16)

    # conv_w^T fp16, duplicated for column packing:
    # wT2 [128(d), cc, 2, 64] with both 64-col halves == w^T chunk
    wnat = const.tile([64, D], F32, tag="wnat")
    nc.scalar.dma_start(wnat, w)
    w16 = const.tile([64, D], F16, tag="w16")
    nc.vector.tensor_copy(w16, wnat)
    wT_ps = ps_s.tile([128, DC, K], F16, tag="s", name="wTps")
    for cc in range(DC):
        nc.tensor.transpose(
            wT_ps[:, cc, :], w16[:, cc * 128:(cc + 1) * 128],
            ident16[:64, :64],
        )
    wT2 = const.tile([128, DC, 2, K], F16, tag="wT2")
    nc.scalar.copy(wT2[:, :, 0, :], wT_ps)
    nc.vector.tensor_copy(wT2[:, :, 1, :], wT_ps)

    # centers^T fp32, duplicated across the two partition halves:
    # cTT [128(k-pair), DC, 128(d)]
    cnat = const.tile([128, DC, K], F32, tag="cnat")
    nc.scalar.dma_start(cnat, c.rearrange("(cc p) k -> p cc k", p=128))
    cT_ps = ps_et.tile([64, DC, 128], F32, tag="et", name="cTps")
    for cc in range(DC):
        nc.tensor.transpose(cT_ps[:, cc, :], cnat[:, cc, :], ident)
    cTT = const.tile([128, DC, 128], F32, tag="cTT")
    nc.scalar.copy(cTT[0:64], cT_ps)
    nc.vector.tensor_copy(cTT[64:128], cT_ps)
    cTTf = cTT.rearrange("p cc d -> p (cc d)")

    # ---- DMA + fp16 casts, n-half granularity ------------------------
    # x16[b][h] is [128, DC, 512] fp16; DMA order interleaves the two
    # batches of a pair so its h0 halves land first.
    x16s = [[None, None] for _ in range(BC)]

    def emit_load(b, h):
        xb = xpool.tile([128, DC, NH], F32, tag="xnat", name=f"x{b}_{h}")
        nc.sync.dma_start(
            xb, x[b].rearrange("(cc p) n -> p cc n", p=128)[
                :, :, h * NH:(h + 1) * NH]
        )
        xh = x16p.tile([128, DC, NH], F16, tag="x16", name=f"x16_{b}_{h}")
        # three-way cast split so no single engine carries it
        nc.vector.tensor_copy(xh[:, :, 0:256], xb[:, :, 0:256])
        nc.scalar.copy(xh[:, :, 256:384], xb[:, :, 256:384])
        nc.gpsimd.tensor_copy(xh[:, :, 384:NH], xb[:, :, 384:NH])
        x16s[b][h] = xh

    desc_v = out.rearrange("b (cc p k) -> p cc b k", cc=DC, p=128, k=K)

    # per-pair state carried across the software pipeline
    state = {}

    def emit_scores_half(p, h):
        """Col-packed scores + exp + E^T + softmax sums for half h."""
        st = state.setdefault(p, {})
        if "E16" not in st:
            st["E16"] = epool.tile([128, 2, 512], F16, tag="E", name=f"E{p}")
            st["AT"] = apool.tile([128, NB, 128], F16, tag="AT", name=f"AT{p}")
            st["red"] = spool.tile([128, NB, 2], F32, tag="red", name=f"rd{p}")
            st["rec"] = spool.tile([128, NB, 2], F32, tag="rec", name=f"rc{p}")
            st["rec16"] = spool.tile([128, NB, 2], F16, tag="rec16",
                                     name=f"rc16{p}")
        E16 = st["E16"]
        xa, xb = x16s[2 * p][h], x16s[2 * p + 1][h]
        sa = ps_s.tile([128, 512], F32, tag="s", name=f"sa{p}_{h}")
        sb = ps_s.tile([128, 512], F32, tag="s", name=f"sb{p}_{h}")
        for cc in range(DC):
            nc.tensor.matmul(
                sa[0:64, :], lhsT=wT2[:, cc, 0, :], rhs=xa[:, cc, :],
                start=(cc == 0), stop=(cc == DC - 1), tile_position=(0, 0),
            )
            nc.tensor.matmul(
                sb[64:128, :], lhsT=wT2[:, cc, 1, :], rhs=xb[:, cc, :],
                start=(cc == 0), stop=(cc == DC - 1), tile_position=(0, 64),
            )
        nc.scalar.activation(E16[0:64, h, :], sa[0:64, :], func=AF.Exp)
        nc.scalar.activation(E16[64:128, h, :], sb[64:128, :], func=AF.Exp)

        # E^T for this half: chunks j = 4h..4h+3
        et_ps = ps_et.tile([128, 4, 128], F16, tag="et", name=f"et{p}_{h}")
        for jj in range(4):
            nc.tensor.transpose(
                et_ps[:, jj, :], E16[:, h, jj * 128:(jj + 1) * 128], ident16
            )
        j0 = 4 * h
        AT = st["AT"]
        nc.vector.tensor_copy(AT[:, j0:j0 + 4, :], et_ps)

        # softmax sums over k (free dim), per batch half
        red, rec, rec16 = st["red"], st["rec"], st["rec16"]
        nc.vector.tensor_reduce(
            red[:, j0:j0 + 4, :],
            AT[:, j0:j0 + 4, :].rearrange("p j (t k) -> p j t k", t=2),
            axis=AX.X, op=ALU.add,
        )
        nc.vector.reciprocal(rec[:, j0:j0 + 4, :], red[:, j0:j0 + 4, :])
        nc.vector.tensor_copy(rec16[:, j0:j0 + 4, :], rec[:, j0:j0 + 4, :])

    def emit_xt(p, j):
        """x^T transposes + rec-scaled evacuation for pair p, chunk j."""
        st = state[p]
        h, jj = j // 4, j % 4
        xt_ps = ps_xt.tile([128, 2, DC, 128], F16, tag="xt",
                           name=f"xt{p}_{j}")
        for t in range(2):
            xh = x16s[2 * p + t][h]
            for cc in range(DC):
                nc.tensor.transpose(
                    xt_ps[:, t, cc, :], xh[:, cc, jj * 128:(jj + 1) * 128],
                    ident16,
                )
        xt16 = xtp.tile([128, 2, 512], F16, tag="xT", name=f"xt16_{p}_{j}")
        # normalization folded into the evacuation; alternate engines
        rec = st["rec"]
        if j % 2 == 0:
            nc.vector.tensor_scalar_mul(
                xt16[:, 0, :], xt_ps[:, 0], rec[:, j, 0:1])
            nc.scalar.activation(
                xt16[:, 1, :], xt_ps[:, 1], func=AF.Identity,
                scale=rec[:, j, 1:2])
        else:
            nc.scalar.activation(
                xt16[:, 0, :], xt_ps[:, 0], func=AF.Identity,
                scale=rec[:, j, 0:1])
            nc.vector.tensor_scalar_mul(
                xt16[:, 1, :], xt_ps[:, 1], rec[:, j, 1:2])
        st.setdefault("xt16", []).append(xt16)

    def emit_vlad_j(p, j):
        """vlad matmuls for pair p, chunk j (col-packed).

        Column-group a accumulates in its own bank (partitions 0:64 of
        tile va), group b in another (partitions 64:128 of vb).
        """
        st = state[p]
        if j == 0:
            st["va"] = ps_v.tile([128, 512], F32, tag="v", name=f"va{p}")
            st["vb"] = ps_v.tile([128, 512], F32, tag="v", name=f"vb{p}")
        AT, xt16 = st["AT"], st["xt16"][j]
        for t, tp, v_ps in ((0, (0, 0), st["va"]), (1, (0, 64), st["vb"])):
            nc.tensor.matmul(
                v_ps[64 * t:64 * (t + 1), :],
                lhsT=AT[:, j, 64 * t:64 * (t + 1)],
                rhs=xt16[:, t, :],
                start=(j == 0), stop=(j == NB - 1),
                tile_position=tp,
            )

    def emit_as_j(p, j):
        """asum matmuls (assign row-sums) for pair p, chunk j."""
        st = state[p]
        if j == 0:
            st["asa"] = ps_s.tile([128, 1], F32, tag="s", name=f"asa{p}")
            st["asb"] = ps_s.tile([128, 1], F32, tag="s", name=f"asb{p}")
        AT, rec16 = st["AT"], st["rec16"]
        for t, tp, as_ps in ((0, (0, 0), st["asa"]), (1, (0, 64), st["asb"])):
            nc.tensor.matmul(
                as_ps[64 * t:64 * (t + 1), :],
                lhsT=AT[:, j, 64 * t:64 * (t + 1)],
                rhs=rec16[:, j, t:t + 1],
                start=(j == 0), stop=(j == NB - 1),
                tile_position=tp,
            )

    def emit_asum_prep(p):
        """Evacuate asum + precompute the centers term (before vlad ends)."""
        st = state[p]
        asum = spool.tile([128, 1], F32, tag="asum", name=f"asum{p}")
        nc.scalar.copy(asum[0:64, :], st["asa"][0:64, :])
        nc.scalar.copy(asum[64:128, :], st["asb"][64:128, :])
        cs = vpool.tile([128, 512], F32, tag="cs", name=f"cs{p}")
        nc.vector.tensor_scalar_mul(cs, cTTf, asum)
        st["cs"] = cs

    def emit_post(p):
        """Centers correction, intra-norm, transpose back, store (pair)."""
        st = state[p]
        # negV = cTT*asum - vlad^T  (plain ops; scalar_tensor_tensor and
        # tensor_tensor_reduce fault this HW runtime)
        cs = st["cs"]
        negV = vpool.tile([128, 512], F32, tag="negV", name=f"negV{p}")
        nc.vector.tensor_sub(negV[0:64, :], cs[0:64, :], st["va"][0:64, :])
        nc.vector.tensor_sub(negV[64:128, :], cs[64:128, :],
                             st["vb"][64:128, :])
        # ss = sum_d negV^2 (ACT square+accum; Square shares Exp's set)
        sq = vpool.tile([128, 512], F16, tag="sq", name=f"sq{p}")
        ss = spool.tile([128, 1], F32, tag="ss", name=f"ss{p}")
        nc.scalar.activation(sq, negV, func=AF.Square, accum_out=ss)
        # rinv = 1/sqrt(ss) via DVE fast-inverse-sqrt + one Newton step
        # (ACT Rsqrt is banned; Sqrt/Ln would thrash the ACT table set)
        hss = spool.tile([128, 1], F32, tag="hss", name=f"hss{p}")
        nc.vector.tensor_scalar(
            hss, ss, 1e-24, 0.5, op0=ALU.max, op1=ALU.mult)
        ti = spool.tile([128, 1], I32, tag="ti", name=f"ti{p}")
        nc.vector.tensor_scalar(
            ti, hss.bitcast(I32), 1, -1,
            op0=ALU.arith_shift_right, op1=ALU.bitwise_xor)
        y0 = spool.tile([128, 1], F32, tag="y0", name=f"y0{p}")
        nc.vector.tensor_scalar_add(y0.bitcast(I32), ti, _FISR_C + 1)
        # Newton step folded with the -1/8 global-norm factor:
        # Vn = negV*y0*(1.5 - hss*y0^2)*(-1/8) = negV*y0*((hss*y0^2-1.5)/8)
        t1 = spool.tile([128, 1], F32, tag="t1", name=f"t1{p}")
        nc.vector.tensor_mul(t1, y0, y0)
        t2 = spool.tile([128, 1], F32, tag="t2", name=f"t2{p}")
        nc.vector.tensor_mul(t2, t1, hss)
        t3 = spool.tile([128, 1], F32, tag="t3", name=f"t3{p}")
        nc.vector.tensor_scalar(
            t3, t2, 1.5, 0.125, op0=ALU.subtract, op1=ALU.mult)
        Vn = vpool.tile([128, 512], F32, tag="Vn", name=f"Vn{p}")
        nc.vector.tensor_scalar(
            Vn, negV, y0, t3, op0=ALU.mult, op1=ALU.mult,
        )
        # transpose back to [d, k-pair] and store both batches
        o_ps = ps_o.tile([128, DC, 128], F32, tag="o", name=f"o{p}")
        for cc in range(DC):
            nc.tensor.transpose(
                o_ps[:, cc, :], Vn[:, cc * 128:(cc + 1) * 128], ident
            )
        o_sb = opool.tile([128, DC, 128], F32, tag="O", name=f"O{p}")
        nc.scalar.copy(o_sb, o_ps)
        nc.sync.dma_start(desc_v[:, :, 2 * p, :], o_sb[:, :, 0:64])
        nc.sync.dma_start(desc_v[:, :, 2 * p + 1, :], o_sb[:, :, 64:128])

    # ---- software-pipelined emission ---------------------------------
    # DMA order: pair p's two batches' h0 halves, then their h1 halves.
    for p in range(NPAIR):
        emit_load(2 * p, 0)
        emit_load(2 * p + 1, 0)
        emit_load(2 * p, 1)
        emit_load(2 * p + 1, 1)

    for p in range(NPAIR):
        emit_scores_half(p, 0)
        for j in range(4):
            emit_xt(p, j)
            if p >= 1:
                emit_vlad_j(p - 1, 4 + j)
        if p >= 1:
            emit_post(p - 1)
        emit_scores_half(p, 1)
        for j in range(4, NB):
            emit_xt(p, j)
            emit_as_j(p, j - 4)
            emit_as_j(p, j)
            emit_vlad_j(p, j - 4)
        emit_asum_prep(p)
    for j in range(4, NB):
        emit_vlad_j(NPAIR - 1, j)
    emit_post(NPAIR - 1)


_NC_CACHE = None


def _build_nc():
    global _NC_CACHE
    if _NC_CACHE is not None:
        return _NC_CACHE
    from contextlib import ExitStack

    nc = bacc.Bacc("TRN2", target_bir_lowering=False, debug=False,
                   num_devices=NCORES)
    x = nc.dram_tensor("x", [BC, D, N], F32, kind="ExternalInput").ap()
    w = nc.dram_tensor("conv_w", [K, D], F32, kind="ExternalInput").ap()
    c = nc.dram_tensor("centers", [D, K], F32, kind="ExternalInput").ap()
    out = nc.dram_tensor("desc", [BC, D * K], F32, kind="ExternalOutput").ap()
    with tile.TileContext(nc) as tc, ExitStack() as ctx:
        _netvlad_core(ctx, tc, out, x, w, c)
    nc.compile()
    _NC_CACHE = nc
    return nc


def kernel(x, conv_w, centers):
    x = np.ascontiguousarray(x, dtype=np.float32)
    conv_w = np.ascontiguousarray(conv_w, dtype=np.float32)
    centers = np.ascontiguousarray(centers, dtype=np.float32)
    nc = _build_nc()
    in_maps = [
        {
            "x": np.ascontiguousarray(x[i * BC:(i + 1) * BC]),
            "conv_w": conv_w,
            "centers": centers,
        }
        for i in range(NCORES)
    ]
    res = run_bass_kernel_spmd(nc, in_maps, core_ids=list(range(NCORES)))
    return np.concatenate([r["desc"] for r in res.results], axis=0)


# revision 22
# speedup vs baseline: 1.0480x; 1.0480x over previous
"""NetVLAD pooling kernel for Trainium2 (Bass/Tile), 8-core data-parallel.

Reference computation (per batch b):
    scores = conv_w @ x[b]                  # [K, N]
    assign = softmax(scores, axis=K)
    vlad   = x[b] @ assign.T - centers * assign.sum(n)   # [D, K]
    vlad  /= max(||vlad||_2 over D, eps)    # intra-norm per cluster column
    desc   = vlad.reshape(D*K) / max(||.||_2, eps)

Shapes: x [32, 512, 1024] f32, conv_w [64, 512], centers [512, 64],
output desc [32, 32768] f32.  Sharding: data-parallel over batch,
4 batches per core; params replicated.

Per-core layout: batches are processed in PAIRS column-packed into the
128-wide PE array (batch b0 on array columns / output partitions 0:64,
b1 on 64:128 via tile_position), so most matmuls, activations, softmax
reduces and norm-chain ops handle two batches at once.  The matmul
path runs in fp16 (FWL weight loads, 2x DVE evacuations, pairing legal
— fp32r forbids nonzero dst partition); PSUM accumulation and the norm
chain stay fp32.  Each column-group accumulates into its OWN psum bank
(interleaved accumulation groups in one bank are unsafe because
start_tensor_calc's has_written clear granularity is bank-wide).

x is DMA'd and cast to fp16 in n-halves (1 MB granularity) so the
scores/softmax/x^T pipeline starts as soon as the first half-batches
arrive instead of waiting for whole batches.

Softmax normalization is folded into the x^T PSUM->SBUF evacuation
(scale=1/rowsum), so vlad consumes the UNNORMALIZED exp(scores)^T as
the stationary, and the assign row-sums come from an rhs=rec matmul
riding the same stationary.  1/sqrt(ss) is computed on DVE with the
bitcast fast-inverse-sqrt seed + one Newton step (~0.2% worst case,
well inside tolerance); ACT then only ever uses Exp/Square/Copy/
Identity, which share one table set - zero ACT table reloads.
(scalar_tensor_tensor and tensor_tensor_reduce fault this HW runtime -
NRT_EXEC_UNIT_UNRECOVERABLE - so the norm chain sticks to plain ops.)

The second L2 normalization is folded to a constant 1/8: after the
intra-normalization each of the K=64 columns has unit norm, so
||desc|| = 8 up to fp32 rounding.

Emission software-pipelines the two pairs: pair p's x^T transposes
interleave with pair p-1's vlad matmuls so the PE never idles long
enough for the HAM clock gate to re-throttle.
"""

import numpy as np

import concourse.bass as bass
from concourse import bacc
import concourse.mybir as mybir
import concourse.tile as tile
from concourse.bass_utils import run_bass_kernel_spmd
from concourse.masks import make_identity

B, D, K, N = 32, 512, 64, 1024
NCORES = 8
BC = B // NCORES          # batches per core
NPAIR = BC // 2           # batch pairs per core
F32 = mybir.dt.float32
F16 = mybir.dt.float16
I32 = mybir.dt.int32
EPS = 1e-12
DC = D // 128             # d chunks (4)
NB = N // 128             # n chunks (8)
NH = N // 2               # n elements per half

AF = mybir.ActivationFunctionType
ALU = mybir.AluOpType
AX = mybir.AxisListType

# fast-inverse-sqrt seed for rsqrt(2*h): C' = 0x5f3759df - 0x400000
_FISR_C = 0x5EF759DF


def _netvlad_core(ctx, tc, out, x, w, c):
    """Emit the per-core tile program.

    out: desc [BC, D*K] f32 DRAM     x: [BC, D, N] f32 DRAM
    w:   conv_w [K, D] f32 DRAM      c: centers [D, K] f32 DRAM
    """
    nc = tc.nc

    const = ctx.enter_context(tc.tile_pool(name="const", bufs=1))
    xpool = ctx.enter_context(tc.tile_pool(name="xp", bufs=4))
    x16p = ctx.enter_context(tc.tile_pool(name="x16", bufs=8))
    epool = ctx.enter_context(tc.tile_pool(name="ep", bufs=2))
    apool = ctx.enter_context(tc.tile_pool(name="ap", bufs=2))
    spool = ctx.enter_context(tc.tile_pool(name="sp", bufs=4))
    xtp = ctx.enter_context(tc.tile_pool(name="xtp", bufs=16))
    vpool = ctx.enter_context(tc.tile_pool(name="vp", bufs=2))
    opool = ctx.enter_context(tc.tile_pool(name="op", bufs=2))
    # PSUM 8 banks: s(2, also hosts as tiles) + et(1) + xt(2) + v(2) + o(1)
    ps_s = ctx.enter_context(tc.tile_pool(name="pss", bufs=2, space="PSUM"))
    ps_et = ctx.enter_context(tc.tile_pool(name="pset", bufs=1, space="PSUM"))
    ps_xt = ctx.enter_context(tc.tile_pool(name="psxt", bufs=2, space="PSUM"))
    ps_v = ctx.enter_context(tc.tile_pool(name="psv", bufs=2, space="PSUM"))
    ps_o = ctx.enter_context(tc.tile_pool(name="pso", bufs=1, space="PSUM"))

    # ---- constants ----------------------------------------------------
    # (w/c DMAs go out on the ACT hwdge queue so the x loads own sync's)
    ident = const.tile([128, 128], F32, tag="ident")
    make_identity(nc, ident)
    ident16 = const.tile([128, 128], F16, tag="ident16")
    make_identity(nc, ident16)

    # conv_w^T fp16, duplicated for column packing:
    # wT2 [128(d), cc, 2, 64] with both 64-col halves == w^T chunk
    wnat = const.tile([64, D], F32, tag="wnat")
    nc.scalar.dma_start(wnat, w)
    w16 = const.tile([64, D], F16, tag="w16")
    nc.vector.tensor_copy(w16, wnat)
    wT_ps = ps_s.tile([128, DC, K], F16, tag="s", name="wTps")
    for cc in range(DC):
        nc.tensor.transpose(
            wT_ps[:, cc, :], w16[:, cc * 128:(cc + 1) * 128],
            ident16[:64, :64],
        )
    wT2 = const.tile([128, DC, 2, K], F16, tag="wT2")
    nc.scalar.copy(wT2[:, :, 0, :], wT_ps)
    nc.vector.tensor_copy(wT2[:, :, 1, :], wT_ps)

    # centers^T fp32, duplicated across the two partition halves:
    # cTT [128(k-pair), DC, 128(d)]
    cnat = const.tile([128, DC, K], F32, tag="cnat")
    nc.scalar.dma_start(cnat, c.rearrange("(cc p) k -> p cc k", p=128))
    cT_ps = ps_et.tile([64, DC, 128], F32, tag="et", name="cTps")
    for cc in range(DC):
        nc.tensor.transpose(cT_ps[:, cc, :], cnat[:, cc, :], ident)
    cTT = const.tile([128, DC, 128], F32, tag="cTT")
    nc.scalar.copy(cTT[0:64], cT_ps)
    nc.vector.tensor_copy(cTT[64:128], cT_ps)
    cTTf = cTT.rearrange("p cc d -> p (cc d)")

    # ---- DMA + fp16 casts, n-half granularity ------------------------
    # x16[b][h] is [128, DC, 512] fp16; DMA order interleaves the two
    # batches of a pair so its h0 halves land first.
    x16s = [[None, None] for _ in range(BC)]

    def emit_load(b, h):
        xb = xpool.tile([128, DC, NH], F32, tag="xnat", name=f"x{b}_{h}")
        nc.sync.dma_start(
            xb, x[b].rearrange("(cc p) n -> p cc n", p=128)[
                :, :, h * NH:(h + 1) * NH]
        )
        xh = x16p.tile([128, DC, NH], F16, tag="x16", name=f"x16_{b}_{h}")
        # split the cast: DVE 5/8, ACT 3/8 (gpsimd measured ~6x slower)
        nc.vector.tensor_copy(xh[:, :, 0:320], xb[:, :, 0:320])
        nc.scalar.copy(xh[:, :, 320:NH], xb[:, :, 320:NH])
        x16s[b][h] = xh

    desc_v = out.rearrange("b (cc p k) -> p cc b k", cc=DC, p=128, k=K)

    # per-pair state carried across the software pipeline
    state = {}

    def emit_scores_half(p, h):
        """Col-packed scores + exp + E^T + softmax sums for half h."""
        st = state.setdefault(p, {})
        if "E16" not in st:
            st["E16"] = epool.tile([128, 2, 512], F16, tag="E", name=f"E{p}")
            st["AT"] = apool.tile([128, NB, 128], F16, tag="AT", name=f"AT{p}")
            st["red"] = spool.tile([128, NB, 2], F32, tag="red", name=f"rd{p}")
            st["rec"] = spool.tile([128, NB, 2], F32, tag="rec", name=f"rc{p}")
            st["rec16"] = spool.tile([128, NB, 2], F16, tag="rec16",
                                     name=f"rc16{p}")
        E16 = st["E16"]
        xa, xb = x16s[2 * p][h], x16s[2 * p + 1][h]
        sa = ps_s.tile([128, 512], F32, tag="s", name=f"sa{p}_{h}")
        sb = ps_s.tile([128, 512], F32, tag="s", name=f"sb{p}_{h}")
        for cc in range(DC):
            nc.tensor.matmul(
                sa[0:64, :], lhsT=wT2[:, cc, 0, :], rhs=xa[:, cc, :],
                start=(cc == 0), stop=(cc == DC - 1), tile_position=(0, 0),
            )
            nc.tensor.matmul(
                sb[64:128, :], lhsT=wT2[:, cc, 1, :], rhs=xb[:, cc, :],
                start=(cc == 0), stop=(cc == DC - 1), tile_position=(0, 64),
            )
        nc.scalar.activation(E16[0:64, h, :], sa[0:64, :], func=AF.Exp)
        nc.scalar.activation(E16[64:128, h, :], sb[64:128, :], func=AF.Exp)

        # E^T for this half: chunks j = 4h..4h+3
        et_ps = ps_et.tile([128, 4, 128], F16, tag="et", name=f"et{p}_{h}")
        for jj in range(4):
            nc.tensor.transpose(
                et_ps[:, jj, :], E16[:, h, jj * 128:(jj + 1) * 128], ident16
            )
        j0 = 4 * h
        AT = st["AT"]
        nc.vector.tensor_copy(AT[:, j0:j0 + 4, :], et_ps)

        # softmax sums over k (free dim), per batch half
        red, rec, rec16 = st["red"], st["rec"], st["rec16"]
        nc.vector.tensor_reduce(
            red[:, j0:j0 + 4, :],
            AT[:, j0:j0 + 4, :].rearrange("p j (t k) -> p j t k", t=2),
            axis=AX.X, op=ALU.add,
        )
        nc.vector.reciprocal(rec[:, j0:j0 + 4, :], red[:, j0:j0 + 4, :])
        nc.vector.tensor_copy(rec16[:, j0:j0 + 4, :], rec[:, j0:j0 + 4, :])

    def emit_xt(p, j):
        """x^T transposes + rec-scaled evacuation for pair p, chunk j."""
        st = state[p]
        h, jj = j // 4, j % 4
        xt_ps = ps_xt.tile([128, 2, DC, 128], F16, tag="xt",
                           name=f"xt{p}_{j}")
        for t in range(2):
            xh = x16s[2 * p + t][h]
            for cc in range(DC):
                nc.tensor.transpose(
                    xt_ps[:, t, cc, :], xh[:, cc, jj * 128:(jj + 1) * 128],
                    ident16,
                )
        xt16 = xtp.tile([128, 2, 512], F16, tag="xT", name=f"xt16_{p}_{j}")
        # normalization folded into the evacuation; alternate engines
        rec = st["rec"]
        if j % 2 == 0:
            nc.vector.tensor_scalar_mul(
                xt16[:, 0, :], xt_ps[:, 0], rec[:, j, 0:1])
            nc.scalar.activation(
                xt16[:, 1, :], xt_ps[:, 1], func=AF.Identity,
                scale=rec[:, j, 1:2])
        else:
            nc.scalar.activation(
                xt16[:, 0, :], xt_ps[:, 0], func=AF.Identity,
                scale=rec[:, j, 0:1])
            nc.vector.tensor_scalar_mul(
                xt16[:, 1, :], xt_ps[:, 1], rec[:, j, 1:2])
        st.setdefault("xt16", []).append(xt16)

    def emit_vlad_j(p, j):
        """vlad matmuls for pair p, chunk j (col-packed).

        Column-group a accumulates in its own bank (partitions 0:64 of
        tile va), group b in another (partitions 64:128 of vb).
        """
        st = state[p]
        if j == 0:
            st["va"] = ps_v.tile([128, 512], F32, tag="v", name=f"va{p}")
            st["vb"] = ps_v.tile([128, 512], F32, tag="v", name=f"vb{p}")
        AT, xt16 = st["AT"], st["xt16"][j]
        for t, tp, v_ps in ((0, (0, 0), st["va"]), (1, (0, 64), st["vb"])):
            nc.tensor.matmul(
                v_ps[64 * t:64 * (t + 1), :],
                lhsT=AT[:, j, 64 * t:64 * (t + 1)],
                rhs=xt16[:, t, :],
                start=(j == 0), stop=(j == NB - 1),
                tile_position=tp,
            )

    def emit_as_j(p, j):
        """asum matmuls (assign row-sums) for pair p, chunk j."""
        st = state[p]
        if j == 0:
            st["asa"] = ps_s.tile([128, 1], F32, tag="s", name=f"asa{p}")
            st["asb"] = ps_s.tile([128, 1], F32, tag="s", name=f"asb{p}")
        AT, rec16 = st["AT"], st["rec16"]
        for t, tp, as_ps in ((0, (0, 0), st["asa"]), (1, (0, 64), st["asb"])):
            nc.tensor.matmul(
                as_ps[64 * t:64 * (t + 1), :],
                lhsT=AT[:, j, 64 * t:64 * (t + 1)],
                rhs=rec16[:, j, t:t + 1],
                start=(j == 0), stop=(j == NB - 1),
                tile_position=tp,
            )

    def emit_asum_prep(p):
        """Evacuate asum + precompute the centers term (before vlad ends)."""
        st = state[p]
        asum = spool.tile([128, 1], F32, tag="asum", name=f"asum{p}")
        nc.scalar.copy(asum[0:64, :], st["asa"][0:64, :])
        nc.scalar.copy(asum[64:128, :], st["asb"][64:128, :])
        cs = vpool.tile([128, 512], F32, tag="cs", name=f"cs{p}")
        nc.vector.tensor_scalar_mul(cs, cTTf, asum)
        st["cs"] = cs

    def emit_post(p):
        """Centers correction, intra-norm, transpose back, store (pair)."""
        st = state[p]
        # negV = cTT*asum - vlad^T  (plain ops; scalar_tensor_tensor and
        # tensor_tensor_reduce fault this HW runtime)
        cs = st["cs"]
        negV = vpool.tile([128, 512], F32, tag="negV", name=f"negV{p}")
        nc.vector.tensor_sub(negV[0:64, :], cs[0:64, :], st["va"][0:64, :])
        nc.vector.tensor_sub(negV[64:128, :], cs[64:128, :],
                             st["vb"][64:128, :])
        # ss = sum_d negV^2 (ACT square+accum; Square shares Exp's set)
        sq = vpool.tile([128, 512], F16, tag="sq", name=f"sq{p}")
        ss = spool.tile([128, 1], F32, tag="ss", name=f"ss{p}")
        nc.scalar.activation(sq, negV, func=AF.Square, accum_out=ss)
        # rinv = 1/sqrt(ss) via DVE fast-inverse-sqrt + one Newton step
        # (ACT Rsqrt is banned; Sqrt/Ln would thrash the ACT table set)
        hss = spool.tile([128, 1], F32, tag="hss", name=f"hss{p}")
        nc.vector.tensor_scalar(
            hss, ss, 1e-24, 0.5, op0=ALU.max, op1=ALU.mult)
        ti = spool.tile([128, 1], I32, tag="ti", name=f"ti{p}")
        nc.vector.tensor_scalar(
            ti, hss.bitcast(I32), 1, -1,
            op0=ALU.arith_shift_right, op1=ALU.bitwise_xor)
        y0 = spool.tile([128, 1], F32, tag="y0", name=f"y0{p}")
        nc.vector.tensor_scalar_add(y0.bitcast(I32), ti, _FISR_C + 1)
        # Newton step folded with the -1/8 global-norm factor:
        # Vn = negV*y0*(1.5 - hss*y0^2)*(-1/8) = negV*y0*((hss*y0^2-1.5)/8)
        t1 = spool.tile([128, 1], F32, tag="t1", name=f"t1{p}")
        nc.vector.tensor_mul(t1, y0, y0)
        t2 = spool.tile([128, 1], F32, tag="t2", name=f"t2{p}")
        nc.vector.tensor_mul(t2, t1, hss)
        t3 = spool.tile([128, 1], F32, tag="t3", name=f"t3{p}")
        nc.vector.tensor_scalar(
            t3, t2, 1.5, 0.125, op0=ALU.subtract, op1=ALU.mult)
        Vn = vpool.tile([128, 512], F32, tag="Vn", name=f"Vn{p}")
        nc.vector.tensor_scalar(
            Vn, negV, y0, t3, op0=ALU.mult, op1=ALU.mult,
        )
        # transpose back to [d, k-pair] and store both batches
        o_ps = ps_o.tile([128, DC, 128], F32, tag="o", name=f"o{p}")
        for cc in range(DC):
            nc.tensor.transpose(
                o_ps[:, cc, :], Vn[:, cc * 128:(cc + 1) * 128], ident
            )
        o_sb = opool.tile([128, DC, 128], F32, tag="O", name=f"O{p}")
        nc.scalar.copy(o_sb, o_ps)
        nc.sync.dma_start(desc_v[:, :, 2 * p, :], o_sb[:, :, 0:64])
        nc.sync.dma_start(desc_v[:, :, 2 * p + 1, :], o_sb[:, :, 64:128])

    # ---- software-pipelined emission ---------------------------------
    # DMA order: pair p's two batches' h0 halves, then their h1 halves.
    for p in range(NPAIR):
        emit_load(2 * p, 0)
        emit_load(2 * p + 1, 0)
        emit_load(2 * p, 1)
        emit_load(2 * p + 1, 1)

    for p in range(NPAIR):
        emit_scores_half(p, 0)
        for j in range(4):
            emit_xt(p, j)
            if p >= 1:
                emit_vlad_j(p - 1, 4 + j)
        if p >= 1:
            emit_post(p - 1)
        emit_scores_half(p, 1)
        for j in range(4, NB):
            emit_xt(p, j)
            emit_as_j(p, j - 4)
            emit_as_j(p, j)
            emit_vlad_j(p, j - 4)
        emit_asum_prep(p)
    for j in range(4, NB):
        emit_vlad_j(NPAIR - 1, j)
    emit_post(NPAIR - 1)


_NC_CACHE = None


def _build_nc():
    global _NC_CACHE
    if _NC_CACHE is not None:
        return _NC_CACHE
    from contextlib import ExitStack

    nc = bacc.Bacc("TRN2", target_bir_lowering=False, debug=False,
                   num_devices=NCORES)
    x = nc.dram_tensor("x", [BC, D, N], F32, kind="ExternalInput").ap()
    w = nc.dram_tensor("conv_w", [K, D], F32, kind="ExternalInput").ap()
    c = nc.dram_tensor("centers", [D, K], F32, kind="ExternalInput").ap()
    out = nc.dram_tensor("desc", [BC, D * K], F32, kind="ExternalOutput").ap()
    with tile.TileContext(nc) as tc, ExitStack() as ctx:
        _netvlad_core(ctx, tc, out, x, w, c)
    nc.compile()
    _NC_CACHE = nc
    return nc


def kernel(x, conv_w, centers):
    x = np.ascontiguousarray(x, dtype=np.float32)
    conv_w = np.ascontiguousarray(conv_w, dtype=np.float32)
    centers = np.ascontiguousarray(centers, dtype=np.float32)
    nc = _build_nc()
    in_maps = [
        {
            "x": np.ascontiguousarray(x[i * BC:(i + 1) * BC]),
            "conv_w": conv_w,
            "centers": centers,
        }
        for i in range(NCORES)
    ]
    res = run_bass_kernel_spmd(nc, in_maps, core_ids=list(range(NCORES)))
    return np.concatenate([r["desc"] for r in res.results], axis=0)
